# revision 13
# baseline (speedup 1.0000x reference)
"""JL-GPT2 attention kernel for 8 TRN2 NeuronCores (Bass/Tile).

Sharding (data + head/tensor parallel, per the sharding hint):
  core c -> batch b = c // 4, head group g = c % 4 (4 heads of the 16).
  W_attn columns are split per head group; S_proj (tiny) is folded into the
  q/k weight slices on the host (associativity change only). W_proj rows are
  split per head group, so each core produces a partial [S, D] output;
  partials are summed per batch while unsharding (row-parallel linear),
  b_proj is added once at the end.

Device algorithm (per core), everything fp32 (matmuls in fp32r):
  1. qT/kT = (Wqkjl chunk).T @ hsT   -> JL-projected q,k feature-major
     [128 = 4 heads x 32, S]
  2. v = hsT_block.T @ Wv            -> natural [S, 4 heads x 64]
     stored with a "ones" column per head (row-sum trick)
  3. per (q-block 512, kv-block 128): scoresT = kT_blk.T @ qT_blk (row-packed
     4 heads on the PE array), exp via ScalarE (scale = 1/8 folded in),
     causal handled by skipping upper blocks + restricting column ranges +
     one triangular mask multiply on the diagonal 128x128 sub-block.
  4. outT[65, q] += v_ext.T @ expT   (row 64 accumulates softmax denominators)
  5. normalize: reciprocal(row 64) -> broadcast across partitions (GpSimd) ->
     multiply; write outT feature-major [128 = 2 heads x 64, 2, S]
  6. partial = outT_chunk.T @ W_proj_chunk  -> [S, D], DMA out.
"""

from contextlib import ExitStack

import numpy as np

B, S, D = 2, 2048, 1024
H, HD, K = 16, 64, 32
HPC = 4  # heads per core
NCORES = 8
DC = D // 128  # 8 contraction chunks
QB = 512  # query block (columns of scoresT)
KVB = 128  # kv block (partitions of scoresT)

TRACE = False
TRACE_KWARGS = {}
LAST_RESULT = None

_cache = {}


def _build(has_qkv_bias):
    import concourse.mybir as mybir
    import concourse.tile as tile
    from concourse import bacc

    f32 = mybir.dt.float32
    f32r = mybir.dt.float32r
    Exp = mybir.ActivationFunctionType.Exp
    Identity = mybir.ActivationFunctionType.Identity

    nc = bacc.Bacc("TRN2", target_bir_lowering=False, debug=False)

    hst_d = nc.dram_tensor("hst", [128, DC, S], f32r, kind="ExternalInput").ap()
    wqk_d = nc.dram_tensor("wqk", [128, DC, 256], f32r, kind="ExternalInput").ap()
    wv_d = nc.dram_tensor("wv", [128, DC, 256], f32r, kind="ExternalInput").ap()
    wp_d = nc.dram_tensor("wp", [128, 2, D], f32r, kind="ExternalInput").ap()
    tri_d = nc.dram_tensor("tri", [128, 128], f32, kind="ExternalInput").ap()
    if has_qkv_bias:
        bqk_d = nc.dram_tensor("bqk", [128, 2], f32, kind="ExternalInput").ap()
        bv_d = nc.dram_tensor("bv", [1, 256], f32r, kind="ExternalInput").ap()
    out_d = nc.dram_tensor("out", [S, D], f32, kind="ExternalOutput").ap()

    with tile.TileContext(nc) as tc, ExitStack() as ctx:
        consts = ctx.enter_context(tc.tile_pool(name="consts", bufs=1))
        big = ctx.enter_context(tc.tile_pool(name="big", bufs=1))
        expp = ctx.enter_context(tc.tile_pool(name="expp", bufs=3))
        smallp = ctx.enter_context(tc.tile_pool(name="smallp", bufs=2))
        projp = ctx.enter_context(tc.tile_pool(name="projp", bufs=3))
        mm_ps = ctx.enter_context(tc.tile_pool(name="mm_ps", bufs=2, space="PSUM"))
        sc_ps = ctx.enter_context(tc.tile_pool(name="sc_ps", bufs=1, space="PSUM"))
        po_ps = ctx.enter_context(tc.tile_pool(name="po_ps", bufs=4, space="PSUM"))

        # ---- input DMAs ----
        hst = big.tile([128, DC, S], f32r, tag="hst")
        for c in range(DC):
            nc.sync.dma_start(hst[:, c], hst_d[:, c])
        wqk = big.tile([128, DC, 256], f32r, tag="wqk")
        nc.sync.dma_start(wqk[:], wqk_d[:])
        wv = big.tile([128, DC, 256], f32r, tag="wv")
        nc.sync.dma_start(wv[:], wv_d[:])
        wp = big.tile([128, 2, D], f32r, tag="wp")
        nc.sync.dma_start(wp[:], wp_d[:])
        tri = consts.tile([128, 128], f32, tag="tri")
        nc.sync.dma_start(tri[:], tri_d[:])
        onesf = consts.tile([128, 1], f32, tag="onesf")
        nc.vector.memset(onesf[:], 1.0)
        ones1 = consts.tile([1, 128], f32r, tag="ones1")
        nc.vector.tensor_copy(ones1[:], onesf[0:1, 0:1].to_broadcast([1, 128]))
        if has_qkv_bias:
            bqk = consts.tile([128, 2], f32, tag="bqk")
            nc.sync.dma_start(bqk[:], bqk_d[:])
            bv = consts.tile([1, 256], f32r, tag="bv")
            nc.sync.dma_start(bv[:], bv_d[:])

        qkT = big.tile([128, 2, S], f32r, tag="qkT")  # [4h x 32, {q,k}, S]
        vext = big.tile([128, S // KVB, HPC * 65], f32r, tag="vext")
        vext_r = vext[:].rearrange("p b (h e) -> p b h e", e=65)
        nc.vector.tensor_copy(  # ones column per head
            vext_r[:, :, :, 64:65],
            onesf[:, None, None, :].to_broadcast([128, S // KVB, HPC, 1]),
        )
        outT = big.tile([128, 2, S], f32r, tag="outT")  # [2h x 64, pair, S]

        # ---- stage A: qT / kT (feature-major) ----
        for t in range(2):  # 0 = q, 1 = k
            for sb in range(S // QB):
                ps = mm_ps.tile([128, QB], f32, tag="mm")
                for c in range(DC):
                    nc.tensor.matmul(
                        ps[:],
                        lhsT=wqk[:, c, t * 128 : (t + 1) * 128],
                        rhs=hst[:, c, sb * QB : (sb + 1) * QB],
                        start=(c == 0),
                        stop=(c == DC - 1),
                    )
                dst = qkT[:, t, sb * QB : (sb + 1) * QB]
                if has_qkv_bias:
                    nc.scalar.activation(dst, ps[:], Identity, bias=bqk[:, t : t + 1])
                else:
                    nc.vector.tensor_copy(dst, ps[:])

        # ---- stage B: v (natural layout, with ones column) ----
        for sb in range(S // KVB):
            ps = mm_ps.tile([128, 256], f32, tag="mm")
            for c in range(DC):
                nc.tensor.matmul(
                    ps[:],
                    lhsT=hst[:, c, sb * KVB : (sb + 1) * KVB],
                    rhs=wv[:, c, :],
                    start=(c == 0),
                    stop=(c == DC - 1 and not has_qkv_bias),
                )
            if has_qkv_bias:
                nc.tensor.matmul(
                    ps[:],
                    lhsT=ones1[0:1, :],
                    rhs=bv[0:1, :],
                    start=False,
                    stop=True,
                )
            nc.vector.tensor_copy(
                vext_r[:, sb, :, 0:64], ps[:].rearrange("p (h e) -> p h e", e=64)
            )

        # ---- stage C: attention (transposed, causal) ----
        tri_bc = tri[:, None, :].to_broadcast([128, 2, 128])
        for qb in range(S // QB):
            c0 = qb * QB
            nkv = qb * (QB // KVB) + (QB // KVB)
            po = [
                po_ps.tile([65, QB], f32, tag="po", name=f"po{qb}_{h}")
                for h in range(HPC)
            ]
            for r in range(nkv):
                o = r * KVB - c0  # diagonal offset of this kv block
                lo = max(o, 0)
                w = QB - lo
                for pair in range(2):
                    ps = sc_ps.tile([128, 2 * QB], f32, tag="sc")
                    ps_r = ps[:].rearrange("p (g q) -> p g q", q=QB)
                    et = expp.tile([128, 2, QB], f32r, tag="exp")
                    for hh in range(2):
                        h = pair * 2 + hh
                        nc.tensor.matmul(
                            ps_r[:, hh, :],
                            lhsT=qkT[
                                h * 32 : (h + 1) * 32, 1, r * KVB : (r + 1) * KVB
                            ],
                            rhs=qkT[h * 32 : (h + 1) * 32, 0, c0 : c0 + QB],
                            start=True,
                            stop=True,
                            tile_position=(32 * h, 0),
                        )
                    # exp(score / sqrt(HD)); no max-subtraction needed (scores
                    # are O(5) by construction, exp cannot overflow fp32)
                    nc.scalar.activation(
                        et[:, :, lo:], ps_r[:, :, lo:], Exp, scale=1.0 / np.sqrt(HD)
                    )
                    if o >= 0:
                        # mask the triangular 128-wide diagonal band
                        nc.vector.tensor_mul(
                            et[:, :, o : o + 128], et[:, :, o : o + 128], tri_bc
                        )
                    for hh in range(2):
                        h = pair * 2 + hh
                        nc.tensor.matmul(
                            po[h][:, lo:],
                            lhsT=vext_r[:, r, h, :],
                            rhs=et[:, hh, lo:],
                            start=(r == 0),
                            stop=(r == nkv - 1),
                        )
            for h in range(HPC):
                rc = smallp.tile([1, QB], f32r, tag="rc")
                with nc.allow_low_precision(reason="softmax denominators are fp32r for the PE broadcast"):
                    nc.vector.reciprocal(rc[:], po[h][64:65, :])
                # broadcast 1/sum across 64 partitions via a K=1 outer product
                pb = mm_ps.tile([64, QB], f32, tag="mm", name=f"pb{qb}_{h}")
                nc.tensor.matmul(
                    pb[:],
                    lhsT=ones1[0:1, 0:64],
                    rhs=rc[0:1, :],
                    start=True,
                    stop=True,
                )
                bc = smallp.tile([64, QB], f32, tag="bc")
                nc.vector.tensor_copy(bc[:], pb[:])
                nc.vector.tensor_mul(
                    outT[(h % 2) * 64 : (h % 2) * 64 + 64, h // 2, c0 : c0 + QB],
                    po[h][0:64, :],
                    bc[:],
                )

        # ---- stage D: output projection (partial sum over this head group) ----
        for sb in range(S // 128):
            for nh in range(2):
                pp = mm_ps.tile([128, 512], f32, tag="mm")
                for j in range(2):
                    nc.tensor.matmul(
                        pp[:],
                        lhsT=outT[:, j, sb * 128 : (sb + 1) * 128],
                        rhs=wp[:, j, nh * 512 : (nh + 1) * 512],
                        start=(j == 0),
                        stop=(j == 1),
                    )
                ot = projp.tile([128, 512], f32, tag="projout")
                if (sb * 2 + nh) % 2 == 0:
                    nc.scalar.copy(ot[:], pp[:])
                else:
                    nc.vector.tensor_copy(ot[:], pp[:])
                nc.sync.dma_start(
                    out_d[sb * 128 : (sb + 1) * 128, nh * 512 : (nh + 1) * 512], ot[:]
                )

    nc.finalize()
    return nc


def _get_nc(has_qkv_bias):
    key = bool(has_qkv_bias)
    if key not in _cache:
        _cache[key] = _build(key)
    return _cache[key]


def kernel(hidden_states, W_attn, b_attn, S_proj, W_proj, b_proj):
    global LAST_RESULT
    from concourse.bass_utils import run_bass_kernel_spmd

    hs = np.asarray(hidden_states, np.float32)
    W_attn = np.asarray(W_attn, np.float32)
    b_attn = np.asarray(b_attn, np.float32)
    S_proj = np.asarray(S_proj, np.float32)
    W_proj = np.asarray(W_proj, np.float32)
    b_proj = np.asarray(b_proj, np.float32)

    has_bias = bool(np.any(b_attn))
    nc = _get_nc(has_bias)

    SpT = S_proj.T  # [HD, K]
    tri = np.triu(np.ones((128, 128), np.float32))  # valid where q_local >= kv_local
    GW = HPC * HD  # 256 columns per head group

    def sw(a, chunks, width):  # [D-like, width] -> [128, chunks, width] SBUF layout
        return np.ascontiguousarray(a.reshape(chunks, 128, width).transpose(1, 0, 2))

    in_maps = []
    for core in range(NCORES):
        b, g = divmod(core, 4)
        Wq = W_attn[:, 0 * D + g * GW : 0 * D + (g + 1) * GW]
        Wk = W_attn[:, 1 * D + g * GW : 1 * D + (g + 1) * GW]
        Wv = W_attn[:, 2 * D + g * GW : 2 * D + (g + 1) * GW]
        Wq_jl = (Wq.reshape(D, HPC, HD) @ SpT).reshape(D, HPC * K)
        Wk_jl = (Wk.reshape(D, HPC, HD) @ SpT).reshape(D, HPC * K)
        wqk = np.concatenate([Wq_jl, Wk_jl], axis=1)  # [D, 256]
        m = {
            "hst": sw(hs[b].T, DC, S),
            "wqk": sw(wqk, DC, 256),
            "wv": sw(Wv, DC, 256),
            "wp": sw(W_proj[g * GW : (g + 1) * GW, :], 2, D),
            "tri": tri,
        }
        if has_bias:
            bq = b_attn[0 * D + g * GW : 0 * D + (g + 1) * GW]
            bk = b_attn[1 * D + g * GW : 1 * D + (g + 1) * GW]
            bv = b_attn[2 * D + g * GW : 2 * D + (g + 1) * GW]
            bq_jl = (bq.reshape(HPC, HD) @ SpT).reshape(HPC * K)
            bk_jl = (bk.reshape(HPC, HD) @ SpT).reshape(HPC * K)
            m["bqk"] = np.ascontiguousarray(np.stack([bq_jl, bk_jl], axis=1))
            m["bv"] = np.ascontiguousarray(bv.reshape(1, GW))
        in_maps.append(m)

    res = run_bass_kernel_spmd(
        nc,
        in_maps,
        core_ids=list(range(NCORES)),
        trace=TRACE,
        **TRACE_KWARGS,
    )
    LAST_RESULT = res

    out = np.zeros((B, S, D), np.float32)
    for core in range(NCORES):
        b, _ = divmod(core, 4)
        out[b] += res.results[core]["out"]
    out += b_proj
    return out
